# revision 1
# baseline (speedup 1.0000x reference)
"""GCN (2-layer + segment-max pool + linear head) on 8 TRN2 NeuronCores.

Sharding: nodes are relabeled into per-graph slots (stride S, 8 graphs per
core) so every core owns a contiguous slot range and all program structure
(windows, buckets, pooling segments) is uniform across cores — one SPMD
program. Edges live with their destination core. Message passing:
  h1s = (x @ W1) * dinv   (bf16 table, allgathered)
  per-edge dma_gather of 256B table rows (int16 index groups of 28672)
  PE aggregation: psum[dst_window] += sel.T @ msgs, sel = onehot(dst)*w
  out = relu(dinv * psum + b)
Pooling is a fixed-stride reduce_max per graph; final head is a tiny matmul.
"""

import math

import numpy as np

N_NODES = 100000
N_EDGES = 3200000
NUM_GRAPHS = 64
IN_DIM = 256
HID = 64
HID2 = 32
NCORES = 8
GRAPHS_PER_CORE = NUM_GRAPHS // NCORES
GROUP = 28672  # int16-indexable gather group (< 32768)
P = 128
CALL_CHUNKS = 8  # max chunks (1024 idxs) per dma_gather call
NEG = -1.0e30


def _host_prep(x, edge_index, edge_weight, batch):
    """Relabel nodes into slots, bucket edges, build all per-core arrays."""
    batch = np.asarray(batch).astype(np.int64)
    counts = np.bincount(batch, minlength=NUM_GRAPHS)
    cum = np.zeros(NUM_GRAPHS + 1, dtype=np.int64)
    np.cumsum(counts, out=cum[1:])
    S = int(math.ceil(max(1, counts.max()) / P) * P)
    NSC = GRAPHS_PER_CORE * S  # slots per core
    NW = NSC // P  # windows per core
    NT = NCORES * NSC  # total slots
    ngroups = int(math.ceil(NT / GROUP))

    nodes = np.arange(N_NODES, dtype=np.int64)
    slot_of = batch * S + (nodes - cum[batch])
    node_of_slot = np.full(NT, -1, dtype=np.int64)
    node_of_slot[slot_of] = nodes

    src = np.asarray(edge_index[0]).astype(np.int64)
    dst = np.asarray(edge_index[1]).astype(np.int64)
    w = np.asarray(edge_weight).astype(np.float32)
    src_all = np.concatenate([src, nodes])
    dst_all = np.concatenate([dst, nodes])
    w_all = np.concatenate([w, np.ones(N_NODES, np.float32)])

    ss = slot_of[src_all]
    ds = slot_of[dst_all]
    core = ds // NSC
    dloc = ds - core * NSC
    win = dloc // P
    dwin = (dloc % P).astype(np.int32)
    grp = ss // GROUP
    sloc = (ss - grp * GROUP).astype(np.int32)

    NB = NW * ngroups  # buckets per core
    bucket = win * ngroups + grp

    # order edges by (core, bucket, sloc)
    order = np.lexsort((sloc, bucket, core))
    core_s = core[order]
    bucket_s = bucket[order]
    sloc_s = sloc[order]
    dwin_s = dwin[order]
    w_s = w_all[order]

    core_starts = np.searchsorted(core_s, np.arange(NCORES + 1))

    # bucket sizes per core -> uniform chunk counts (max over cores)
    bsizes = np.zeros((NCORES, NB), dtype=np.int64)
    for c in range(NCORES):
        lo, hi = core_starts[c], core_starts[c + 1]
        bsizes[c] = np.bincount(bucket_s[lo:hi], minlength=NB)
    bchunks = np.maximum(1, np.ceil(bsizes.max(axis=0) / P).astype(np.int64))
    cstart = np.zeros(NB + 1, dtype=np.int64)
    np.cumsum(bchunks, out=cstart[1:])
    TCH = int(cstart[NB])  # total chunks (uniform)

    # in-degree (weights) per slot, padded node-major layout for deg reduce
    indeg = np.bincount(ds, minlength=NT)
    K = int(indeg.max())
    ds_order = np.argsort(ds, kind="stable")
    ds_sorted = ds[ds_order]
    w_for_deg = w_all[ds_order]
    slot_first = np.searchsorted(ds_sorted, np.arange(NT))
    rank = np.arange(len(ds_sorted)) - slot_first[ds_sorted]
    wslot_all = np.zeros((NT, K), dtype=np.float32)
    wslot_all[ds_sorted, rank] = w_for_deg
    dummy = node_of_slot < 0
    wslot_all[dummy, 0] = 1.0  # deg=1 for dummies -> dinv finite

    import ml_dtypes

    bf16 = ml_dtypes.bfloat16
    x = np.asarray(x).astype(np.float32)

    per_core = []
    meta = dict(S=S, NSC=NSC, NW=NW, NT=NT, ngroups=ngroups, NB=NB,
                TCH=TCH, K=K, bchunks=bchunks, cstart=cstart)
    for c in range(NCORES):
        lo, hi = core_starts[c], core_starts[c + 1]
        bs = bucket_s[lo:hi]
        sl = sloc_s[lo:hi]
        dw = dwin_s[lo:hi]
        ww = w_s[lo:hi]
        # position of each edge in the padded chunk space
        b_first = np.searchsorted(bs, np.arange(NB))
        erank = np.arange(hi - lo) - b_first[bs]
        pos = cstart[bs] * P + erank  # bucket-padded global position

        gidx = np.zeros(TCH * P, dtype=np.int16)
        gidx[pos] = sl.astype(np.int16)
        sel = np.zeros((TCH, P, P), dtype=np.float32)
        sel[pos // P, pos % P, dw] = ww
        sel = sel.astype(bf16)
        # wrapped int16 index layout: idx i -> partition i%16, col i//16
        idxw = np.ascontiguousarray(
            np.tile(gidx.reshape(-1, 16).T, (8, 1))
        )  # [128, TCH*8]

        # xT with dummy slots zero: [2, 128, NSC]
        nos = node_of_slot[c * NSC:(c + 1) * NSC]
        xs = np.zeros((NSC, IN_DIM), dtype=np.float32)
        real = nos >= 0
        xs[real] = x[nos[real]]
        xT = np.ascontiguousarray(xs.T.reshape(2, P, NSC))

        # wslot node-major [128, NW, K]: slot = w*128 + p
        wsl = wslot_all[c * NSC:(c + 1) * NSC].reshape(NW, P, K)
        wsl = np.ascontiguousarray(wsl.transpose(1, 0, 2))

        dbias = np.where(real, 0.0, NEG).astype(np.float32)
        dbias = np.ascontiguousarray(dbias.reshape(NW, P).T)  # [128, NW]

        per_core.append(dict(idx=idxw, sel=sel, xT=xT, wslot=wsl, dbias=dbias))
    return meta, per_core


def _build_program(meta, reps=1):
    import concourse.bacc as bacc
    import concourse.bass as bass
    import concourse.mybir as mybir
    import concourse.tile as tile
    from concourse.library_config import mlp
    from concourse.masks import make_identity

    S, NSC, NW, NT = meta["S"], meta["NSC"], meta["NW"], meta["NT"]
    ngroups, NB, TCH, K = meta["ngroups"], meta["NB"], meta["TCH"], meta["K"]
    bchunks, cstart = meta["bchunks"], meta["cstart"]
    BF = mybir.dt.bfloat16
    F32 = mybir.dt.float32

    nc = bacc.Bacc("TRN2", target_bir_lowering=False, debug=False,
                   num_devices=NCORES)
    # inputs
    t_idx = nc.dram_tensor("idx", [P, TCH * 8], mybir.dt.int16,
                           kind="ExternalInput")
    t_sel = nc.dram_tensor("sel", [TCH, P, P], BF, kind="ExternalInput")
    t_xT = nc.dram_tensor("xT", [2, P, NSC], F32, kind="ExternalInput")
    t_wslot = nc.dram_tensor("wslot", [P, NW, K], F32, kind="ExternalInput")
    t_dbias = nc.dram_tensor("dbias", [P, NW], F32, kind="ExternalInput")
    t_W1 = nc.dram_tensor("W1", [2, P, HID], F32, kind="ExternalInput")
    t_b1 = nc.dram_tensor("b1", [HID, 1], F32, kind="ExternalInput")
    t_W2 = nc.dram_tensor("W2", [HID, HID2], F32, kind="ExternalInput")
    t_b2 = nc.dram_tensor("b2", [HID2, 1], F32, kind="ExternalInput")
    t_Wlin = nc.dram_tensor("Wlin", [HID2, 4], F32, kind="ExternalInput")
    t_blin = nc.dram_tensor("blin", [GRAPHS_PER_CORE, 4], F32,
                            kind="ExternalInput")
    t_out = nc.dram_tensor("out", [GRAPHS_PER_CORE, 4], F32,
                           kind="ExternalOutput")
    # internal DRAM for collectives
    shard1 = nc.dram_tensor("shard1", [NSC, P], BF)
    shard2 = nc.dram_tensor("shard2", [NSC, P], BF)
    full1 = nc.dram_tensor("full1", [NT, P], BF, addr_space="Shared")
    full2 = nc.dram_tensor("full2", [NT, P], BF, addr_space="Shared")

    MAXCH = int(bchunks.max())
    rg = [list(range(NCORES))]

    with tile.TileContext(nc) as tc:
      nc.gpsimd.load_library(mlp)
      for _rep in range(reps):
          with (
              tc.tile_pool(name="const", bufs=1) as constp,
              tc.tile_pool(name="big", bufs=1) as bigp,
              tc.tile_pool(name="wsl", bufs=1) as wslp,
              tc.tile_pool(name="xt", bufs=3) as xtp,
              tc.tile_pool(name="tabt", bufs=1) as tabp,
              tc.tile_pool(name="idxt", bufs=4) as idxp,
              tc.tile_pool(name="selt", bufs=4) as selp,
              tc.tile_pool(name="msgt", bufs=4) as msgp,
              tc.tile_pool(name="ep", bufs=4) as epp,
              tc.tile_pool(name="wps", bufs=4, space="PSUM") as wpsum,
              tc.tile_pool(name="tps", bufs=2, space="PSUM") as tpsum,
              tc.tile_pool(name="fps", bufs=1, space="PSUM") as fpsum,
          ):
              ident = constp.tile([P, P], F32)
              make_identity(nc, ident[:])
              w1t = constp.tile([P, 2, HID], F32)
              nc.sync.dma_start(
                  out=w1t[:], in_=t_W1[:].rearrange("k p h -> p k h"))
              b1t = constp.tile([HID, 1], F32)
              nc.sync.dma_start(out=b1t[:], in_=t_b1[:])
              w2t = constp.tile([HID, HID2], F32)
              nc.sync.dma_start(out=w2t[:], in_=t_W2[:])
              b2t = constp.tile([HID2, 1], F32)
              nc.sync.dma_start(out=b2t[:], in_=t_b2[:])
              wlint = constp.tile([HID2, 4], F32)
              nc.sync.dma_start(out=wlint[:], in_=t_Wlin[:])
              blint = constp.tile([GRAPHS_PER_CORE, 4], F32)
              nc.sync.dma_start(out=blint[:], in_=t_blin[:])
              dbiast = constp.tile([P, NW], F32)
              nc.sync.dma_start(out=dbiast[:], in_=t_dbias[:])

              # ---- deg -> dinv ----
              wst = wslp.tile([P, NW, K], F32)
              nc.sync.dma_start(out=wst[:], in_=t_wslot[:])
              deg = constp.tile([P, NW], F32)
              nc.vector.reduce_sum(deg[:], wst[:], axis=mybir.AxisListType.X)
              rdeg = constp.tile([P, NW], F32)
              nc.vector.reciprocal(rdeg[:], deg[:])
              dinv = constp.tile([P, NW], F32)
              nc.scalar.sqrt(dinv[:], rdeg[:])

              # ---- transform 1: table1 = bf16((x @ W1) * dinv), cols 64.. = 0
              tab = tabp.tile([P, NW, P], BF, tag="table")
              nc.vector.memset(tab[:], 0.0)
              XB = 4  # node-blocks per xT DMA
              for b0 in range(0, NW, XB):
                  nb = min(XB, NW - b0)
                  xt = xtp.tile([P, 2, XB * P], F32)
                  nc.sync.dma_start(
                      out=xt[:, :, :nb * P],
                      in_=t_xT[:].rearrange("k p n -> p k n")[
                          :, :, b0 * P:(b0 + nb) * P],
                  )
                  for bb in range(nb):
                      b = b0 + bb
                      ps = wpsum.tile([P, HID], F32, tag="wps")
                      for kk in range(2):
                          nc.tensor.matmul(
                              ps[:],
                              xt[:, kk, bb * P:(bb + 1) * P],
                              w1t[:, kk, :],
                              start=(kk == 0),
                              stop=(kk == 1),
                          )
                      nc.scalar.activation(
                          tab[:, b, :HID], ps[:],
                          mybir.ActivationFunctionType.Copy,
                          scale=dinv[:, b:b + 1],
                      )
              nc.sync.dma_start(
                  out=shard1[:].rearrange("(w p) d -> p w d", p=P), in_=tab[:]
              )
              nc.gpsimd.collective_compute(
                  "AllGather", mybir.AluOpType.bypass, replica_groups=rg,
                  ins=[shard1[:]], outs=[full1[:]],
              )

              def gather_layer(table_full, fdim):
                  """Returns h_T tile [fdim, NSC] f32 = relu(dinv*agg + b)^T
                  per-window; bias/relu applied by caller spec."""
                  for w in range(NW):
                      ps = wpsum.tile([P, HID], F32, tag="wps")
                      first = True
                      for g in range(ngroups):
                          bkt = w * ngroups + g
                          nch = int(bchunks[bkt])
                          c0 = int(cstart[bkt])
                          it = idxp.tile([P, MAXCH * 8], mybir.dt.int16,
                                         tag="idx")
                          nc.sync.dma_start(
                              out=it[:, :nch * 8],
                              in_=t_idx[:, c0 * 8:(c0 + nch) * 8],
                          )
                          st = selp.tile([P, MAXCH, P], BF, tag="sel")
                          nc.sync.dma_start(
                              out=st[:, :nch, :],
                              in_=t_sel[:].rearrange("t e d -> e t d")[
                                  :, c0:c0 + nch, :],
                          )
                          mt = msgp.tile([P, MAXCH, P], BF, tag="msg")
                          glo = g * GROUP
                          ghi = min(NT, (g + 1) * GROUP)
                          for off in range(0, nch, CALL_CHUNKS):
                              ncall = min(CALL_CHUNKS, nch - off)
                              ni = ncall * P
                              nc.gpsimd.dma_gather(
                                  mt[:, off:off + ncall, :],
                                  table_full[glo:ghi],
                                  it[:, off * 8:off * 8 + ni // 16],
                                  ni, ni, P,
                                  single_packet=True,
                              )
                          for t in range(nch):
                              nc.tensor.matmul(
                                  ps[:, :fdim],
                                  st[:, t, :],
                                  mt[:, t, :fdim],
                                  start=first,
                                  stop=(g == ngroups - 1 and t == nch - 1),
                              )
                              first = False
                      yield w, ps

              # ---- layer 1 gather/aggregate -> h2inT [HID, NSC] f32 ----
              h2inT = bigp.tile([HID, NSC], F32, tag="h2inT")
              for w, ps in gather_layer(full1, HID):
                  t1 = epp.tile([P, HID], F32, tag="ep1")
                  nc.scalar.activation(
                      t1[:], ps[:, :HID], mybir.ActivationFunctionType.Copy,
                      scale=dinv[:, w:w + 1],
                  )
                  tp = tpsum.tile([HID, P], F32, tag="tp")
                  nc.tensor.transpose(tp[:], t1[:], ident[:])
                  nc.scalar.activation(
                      h2inT[:, w * P:(w + 1) * P], tp[:],
                      mybir.ActivationFunctionType.Relu, bias=b1t[:, :1],
                  )

              # ---- transform 2: table2 = bf16((h2in @ W2) * dinv) ----
              tab2 = tabp.tile([P, NW, P], BF, tag="table")
              nc.vector.memset(tab2[:], 0.0)
              for b in range(NW):
                  ps = wpsum.tile([P, HID], F32, tag="wps")
                  nc.tensor.matmul(
                      ps[:, :HID2], h2inT[:, b * P:(b + 1) * P], w2t[:],
                      start=True, stop=True,
                  )
                  nc.scalar.activation(
                      tab2[:, b, :HID2], ps[:, :HID2],
                      mybir.ActivationFunctionType.Copy,
                      scale=dinv[:, b:b + 1],
                  )
              nc.sync.dma_start(
                  out=shard2[:].rearrange("(w p) d -> p w d", p=P), in_=tab2[:]
              )
              nc.gpsimd.collective_compute(
                  "AllGather", mybir.AluOpType.bypass, replica_groups=rg,
                  ins=[shard2[:]], outs=[full2[:]],
              )

              # ---- layer 2 gather/aggregate -> agg2T [HID2, NSC] f32 ----
              agg2T = bigp.tile([HID2, NSC], F32, tag="agg2T")
              for w, ps in gather_layer(full2, HID2):
                  t1 = epp.tile([P, HID2], F32, tag="ep2")
                  nc.scalar.activation(
                      t1[:], ps[:, :HID2],
                      mybir.ActivationFunctionType.Identity,
                      scale=dinv[:, w:w + 1], bias=dbiast[:, w:w + 1],
                  )
                  tp = tpsum.tile([HID2, P], F32, tag="tp")
                  nc.tensor.transpose(tp[:, :], t1[:], ident[:, :])
                  nc.scalar.activation(
                      agg2T[:, w * P:(w + 1) * P], tp[:, :],
                      mybir.ActivationFunctionType.Relu, bias=b2t[:, :1],
                  )

              # ---- pooling + head ----
              pooled = constp.tile([HID2, GRAPHS_PER_CORE], F32)
              for j in range(GRAPHS_PER_CORE):
                  nc.vector.reduce_max(
                      pooled[:, j:j + 1], agg2T[:, j * S:(j + 1) * S],
                      axis=mybir.AxisListType.X,
                  )
              fp = fpsum.tile([GRAPHS_PER_CORE, 4], F32)
              nc.tensor.matmul(fp[:], pooled[:], wlint[:], start=True, stop=True)
              outt = constp.tile([GRAPHS_PER_CORE, 4], F32)
              nc.vector.tensor_add(outt[:], fp[:], blint[:])
              nc.sync.dma_start(out=t_out[:], in_=outt[:])

    nc.compile()
    return nc


class _Runner:
    """Single-build PJRT runner (shard_map over 8 cores) under axon."""

    def __init__(self, nc):
        import jax
        from jax.experimental.shard_map import shard_map
        from jax.sharding import Mesh, NamedSharding, PartitionSpec
        import concourse.mybir as mybir
        from concourse.bass2jax import (
            _bass_exec_p, install_neuronx_cc_hook, partition_id_tensor,
        )

        install_neuronx_cc_hook()
        self.jax = jax
        partition_name = (
            nc.partition_id_tensor.name if nc.partition_id_tensor else None
        )
        in_names, out_names, out_avals, zero_outs = [], [], [], []
        for alloc in nc.m.functions[0].allocations:
            if not isinstance(alloc, mybir.MemoryLocationSet):
                continue
            name = alloc.memorylocations[0].name
            if alloc.kind == "ExternalInput":
                if name != partition_name:
                    in_names.append(name)
            elif alloc.kind == "ExternalOutput":
                out_names.append(name)
                shape = tuple(alloc.tensor_shape)
                dtype = mybir.dt.np(alloc.dtype)
                out_avals.append(jax.core.ShapedArray(shape, dtype))
                zero_outs.append(np.zeros(shape, dtype))
        self.param_names = list(in_names)
        self.out_names = out_names
        self.out_avals = out_avals
        self.zero_outs = zero_outs
        n_params, n_outs = len(in_names), len(out_avals)
        all_in = in_names + out_names
        if partition_name is not None:
            all_in.append(partition_name)

        def _body(*args):
            operands = list(args)
            if partition_name is not None:
                operands.append(partition_id_tensor())
            return tuple(_bass_exec_p.bind(
                *operands,
                out_avals=tuple(out_avals),
                in_names=tuple(all_in),
                out_names=tuple(out_names),
                lowering_input_output_aliases=(),
                sim_require_finite=False,
                sim_require_nnan=False,
                nc=nc,
            ))

        self.devices = jax.devices()[:NCORES]
        self.mesh = Mesh(np.asarray(self.devices), ("core",))
        spec = PartitionSpec("core")
        self._fn = jax.jit(
            shard_map(
                _body, mesh=self.mesh,
                in_specs=(spec,) * (n_params + n_outs),
                out_specs=(spec,) * n_outs,
                check_rep=False,
            ),
            keep_unused=True,
        )
        self.sharding = NamedSharding(self.mesh, spec)

    def place(self, in_maps):
        args = []
        for name in self.param_names:
            arr = np.concatenate([np.asarray(m[name]) for m in in_maps], axis=0)
            args.append(self.jax.device_put(arr, self.sharding))
        for z in self.zero_outs:
            zz = np.zeros((NCORES * z.shape[0], *z.shape[1:]), z.dtype)
            args.append(self.jax.device_put(zz, self.sharding))
        return args

    def run(self, args):
        outs = self._fn(*args)
        self.jax.block_until_ready(outs)
        return outs

    def result(self, outs, name):
        i = self.out_names.index(name)
        return np.asarray(outs[i])


_CACHE = {}


def _get_runner(meta):
    key = (meta["S"], meta["TCH"], meta["K"], tuple(meta["bchunks"].tolist()))
    if key not in _CACHE:
        nc = _build_program(meta)
        _CACHE[key] = _Runner(nc)
    return _CACHE[key]


def kernel(x, edge_index, edge_weight, batch, W1, b1, W2, b2, Wlin, blin,
           _timing=None):
    meta, per_core = _host_prep(x, edge_index, edge_weight, batch)
    runner = _get_runner(meta)

    W1a = np.ascontiguousarray(
        np.asarray(W1, dtype=np.float32).reshape(2, P, HID))
    b1a = np.asarray(b1, dtype=np.float32).reshape(HID, 1)
    W2a = np.asarray(W2, dtype=np.float32)
    b2a = np.asarray(b2, dtype=np.float32).reshape(HID2, 1)
    Wlina = np.asarray(Wlin, dtype=np.float32)
    blina = np.tile(np.asarray(blin, dtype=np.float32)[None, :],
                    (GRAPHS_PER_CORE, 1))

    in_maps = []
    for c in range(NCORES):
        d = per_core[c]
        in_maps.append({
            "idx": d["idx"], "sel": d["sel"], "xT": d["xT"],
            "wslot": d["wslot"], "dbias": d["dbias"],
            "W1": W1a, "b1": b1a, "W2": W2a, "b2": b2a,
            "Wlin": Wlina, "blin": blina,
        })
    args = runner.place(in_maps)
    outs = runner.run(args)
    if _timing is not None:
        import time
        for _ in range(_timing.get("warmup", 2)):
            runner.run(args)
        ts = []
        for _ in range(_timing.get("iters", 8)):
            t0 = time.perf_counter()
            runner.run(args)
            ts.append(time.perf_counter() - t0)
        _timing["times"] = ts
    res = runner.result(outs, "out")  # [8*8, 4]
    return res.reshape(NUM_GRAPHS, 4)

